# revision 33
# baseline (speedup 1.0000x reference)
"""Trainium2 Bass kernel for LowRankRayTracer.

csi[f] = (delta_t/D) * v_f^T M v_f,  M = conj(rad)^T conj(att)  (R=32, complex)
contracted over N = D*K = 524288 rows.

Strategy (8 cores):
  - Shard the N rows across cores (512 directions each). csi is linear in M,
    so each core computes its partial S = rad^T att (f32 view of complex
    pairs -> all four real cross products at once), folds S into
    W = [W_real | W_imag], computes partial csi over ALL F=8192 subcarriers,
    and the host just sums the 8 partial csi vectors.
  - Inputs are cast to plain fp16 on the host (tolerance is 2e-2; fp16
    rounding lands ~5e-4 after the 524288-term stochastic accumulation).
    This halves HBM traffic vs an exact hi/lo split AND cuts PE work 3x:
    each 128-row slice is one LDWEIGHTS(128) + one 128-col matmul.
  - Each matmul packs 2 rows per partition: lhsT = rad[:, s*128:+128]
    (2 rows of 64 per partition), rhs = att same slice. PSUM accumulates
    [128,128] where blocks (0:64,0:64) and (64:128,64:128) are the valid
    row_a*row_a and row_b*row_b partial sums (cross blocks are garbage,
    discarded by the fold). Matmuls round-robin over 2 PSUM banks.
  - sqrt(delta_t/D) = 5/256 exactly, folded into g on the host, so W needs
    no scaling pass. W is built directly in fp16 by 8 strided DVE/Pool ops.
    Phase 3 keeps everything fp16 (1 cyc/col on the PE; f32 rhs would run
    at 4 cyc/col as two half-speed passes). e = g .* (W^T g) tiles stream
    to DRAM in a few grouped dma_starts; the host does the final column
    sums (linear, so partial-per-core results just add).
"""

import numpy as np

D, K, R = 4096, 128, 32
F = 8192
N_CORES = 8
DIR_PER_CORE = D // N_CORES              # 512
N_MACRO = 8                              # macro tiles per tensor per core
MACRO_COLS = 4096                        # fp16 per partition per macro tile
SLICE = 128                              # matmul slice width (2 rows/partition)
SCALE = (200.0 / K) / D                  # delta_t / num_directions
GSCALE = 5.0 / 256.0                     # exact sqrt(SCALE)
FCHUNK = 512                             # phase-3 subcarriers per chunk
N_FCHUNK = F // FCHUNK                   # 16
NB = 2                                   # round-robin PSUM accumulator banks

_NC_CACHE = {}


def _build_consts():
    """(128, 256) f32 selection matrices.

    S64[r, f] := acc[r, f] + acc[64+r, 64+f] (fold of the two valid blocks).
    v1 = C1^T acc[:,0:64] + C2^T acc[:,64:128]: v1[0:32] = even rows of S64,
    v1[32:64] = odd rows. v2 (C1x/C2x) is the even/odd swap.
    """
    c = np.zeros((128, 256), np.float32)
    for a in range(32):
        c[2 * a, a] = 1.0                # C1: even rows -> partitions 0:32
        c[2 * a + 1, 32 + a] = 1.0       #     odd rows  -> partitions 32:64
        c[64 + 2 * a, 64 + a] = 1.0      # C2: same for the b-half of acc
        c[64 + 2 * a + 1, 64 + 32 + a] = 1.0
        c[2 * a, 128 + 32 + a] = 1.0     # C1x: swapped
        c[2 * a + 1, 128 + a] = 1.0
        c[64 + 2 * a, 192 + 32 + a] = 1.0
        c[64 + 2 * a + 1, 192 + a] = 1.0
    return c


def build_nc(n_macro=N_MACRO):
    import concourse.bacc as bacc
    import concourse.mybir as mybir
    import concourse.tile as tile

    fp32 = mybir.dt.float32
    fp16 = mybir.dt.float16
    mult = mybir.AluOpType.mult
    sub_ = mybir.AluOpType.subtract
    nc = bacc.Bacc(trn_type="TRN2", target_bir_lowering=False, debug=False)

    rad_d = nc.dram_tensor("rad", [4, 128, 2 * MACRO_COLS], fp16,
                           kind="ExternalInput").ap()
    att_d = nc.dram_tensor("att", [4, 128, 2 * MACRO_COLS], fp16,
                           kind="ExternalInput").ap()
    gth_d = nc.dram_tensor("gth", [64, F], fp16, kind="ExternalInput").ap()
    cst_d = nc.dram_tensor("consts", [128, 256], fp32, kind="ExternalInput").ap()
    out_d = nc.dram_tensor("eout", [128, F], fp16, kind="ExternalOutput").ap()

    # main-loop streaming plan over the [4, 128, 8192] layout: size-ramped
    # chunks from one bufs=3 pool (slots sized for the largest chunk). The
    # per-queue DMA round-robins across pending transfers, so small early
    # chunks land fast for an early PE start, fat middle chunks keep the
    # ~300 GB/s/core HBM ceiling saturated, and tiny last chunks let the PE
    # drain right behind the final bytes. Widths never cross the 8192-col
    # dram tile boundary. First three chunks' att loads go on SP — Act's
    # preamble ACT_TABLE_LOAD would hold them ~8 us at startup.
    widths = [1024, 1024, 2048, 4096, 8192, 8192,
              4096, 2048, 1024, 512, 512]
    assert sum(widths) == 32768
    chunks = []
    pos = 0
    for w in widths:
        chunks.append((pos // 8192, pos % 8192, w))
        pos += w
    total = (MACRO_COLS // SLICE) * n_macro                    # 256 slices

    with tile.TileContext(nc) as tc:
        with (
            tc.tile_pool(name="io", bufs=3) as io_pool,
            tc.tile_pool(name="small", bufs=1) as small,
            tc.tile_pool(name="epool", bufs=8) as epool,
            tc.tile_pool(name="tsb", bufs=3) as tsb_pool,
        ):
            c_sb = small.tile([128, 256], fp32, tag="consts")
            gth2 = small.tile([128, F], fp16, tag="gth2")

            # ---- main loop: S += rad^T att, plain fp16, 128-wide slices ----
            acc = small.tile([128, 128], fp32, tag="acc")
            with tc.tile_pool(name="spsum", bufs=1, space="PSUM") as spsum:
                # full-bank tiles so the accumulators live in separate banks
                banks = [spsum.tile([128, 512], fp32, tag=f"s{b}",
                                    name=f"sbank{b}")
                         for b in range(NB)]
                wscr = spsum.tile([128, 512], fp32, tag="wscr", name="wscr")
                seen = [False] * NB
                idx = 0
                for ci, (ti, c0, w) in enumerate(chunks):
                    if ci == 2:
                        nc.sync.dma_start(c_sb[:], cst_d[:])
                    if ci == 6:
                        # g overlaps the mid-stream chunks, done before phase
                        # 3; the upper half is an SBUF->SBUF duplicate, which
                        # costs queue time but no HBM bandwidth
                        nc.sync.dma_start(gth2[0:64, :], gth_d[:])
                        nc.scalar.dma_start(gth2[64:128, :], gth2[0:64, :])
                    rad = io_pool.tile([128, 8192], fp16, tag="rad")
                    att = io_pool.tile([128, 8192], fp16, tag="att")
                    nc.sync.dma_start(rad[:, 0:w], rad_d[ti, :, c0:c0 + w])
                    att_eng = nc.sync if ci < 3 else nc.scalar
                    att_eng.dma_start(att[:, 0:w], att_d[ti, :, c0:c0 + w])
                    for s in range(w // SLICE):
                        sl = slice(s * SLICE, (s + 1) * SLICE)
                        b = idx % NB
                        nc.tensor.matmul(
                            banks[b][:, 0:128],
                            lhsT=rad[:, sl],
                            rhs=att[:, sl],
                            start=not seen[b],
                            stop=(idx >= total - NB),
                        )
                        seen[b] = True
                        idx += 1

                # pre-warms: in the in-order PE queue these run right after
                # the last main matmul, bridging the bank-sum gap so the
                # clock-ramp counter keeps running into the tail (DMA is
                # idle there, so the PE-boost/HBM power tradeoff is moot)
                for k in range(8):
                    nc.tensor.matmul(wscr[0:64, 0:64], lhsT=c_sb[:, 0:64],
                                     rhs=c_sb[:, 0:64], start=True, stop=True)

                # acc = sum of the accumulator banks
                nc.vector.tensor_copy(acc[:], banks[0][:, 0:128])
                for b in range(1, NB):
                    nc.vector.tensor_add(acc[:], acc[:], banks[b][:, 0:128])

            # ---- epilogue: fold + de-interleave via selection matmuls ----
            with tc.tile_pool(name="vpsum", bufs=1, space="PSUM") as vpsum:
                v1 = vpsum.tile([64, 64], fp32, tag="v1")
                nc.tensor.matmul(v1[:], lhsT=c_sb[:, 0:64], rhs=acc[:, 0:64],
                                 start=True, stop=False)
                nc.tensor.matmul(v1[:], lhsT=c_sb[:, 64:128],
                                 rhs=acc[:, 64:128], start=False, stop=True)
                v2 = vpsum.tile([64, 64], fp32, tag="v2")
                nc.tensor.matmul(v2[:], lhsT=c_sb[:, 128:192],
                                 rhs=acc[:, 0:64], start=True, stop=False)
                nc.tensor.matmul(v2[:], lhsT=c_sb[:, 192:256],
                                 rhs=acc[:, 64:128], start=False, stop=True)

                v1s = small.tile([64, 64], fp32, tag="v1s")
                nc.scalar.copy(v1s[:], v1[:])
                v2s = small.tile([64, 64], fp32, tag="v2s")
                nc.scalar.copy(v2s[:], v2[:])

            # ---- build W = [W_real | W_imag] (64,128) directly in fp16 ----
            # Mr[a,b] = S64[2a,2b]-S64[2a+1,2b+1], Mi = -(S64[2a,2b+1]+S64[2a+1,2b])
            # W_real = [[Mr, -Mi], [-Mi, -Mr]], W_imag = [[Mi, Mr], [Mr, -Mi]]
            # v1[0:32]=even rows, v1[32:64]=odd; v2 swapped. Scale is folded
            # into g on the host (GSCALE^2 == SCALE).
            wh = small.tile([64, 128], fp16, tag="wh")
            E, O = slice(0, 64, 2), slice(1, 64, 2)
            t, b = slice(0, 32), slice(32, 64)
            # top rows: Mr | -Mi(=mp) | Mi | Mr
            nc.vector.tensor_sub(wh[t, 0:32], v1s[t, E], v2s[t, O])
            nc.vector.tensor_add(wh[t, 32:64], v1s[t, O], v2s[t, E])
            nc.vector.scalar_tensor_tensor(wh[t, 64:96], v1s[t, O], -1.0,
                                           v2s[t, E], op0=mult, op1=sub_)
            nc.gpsimd.tensor_sub(wh[t, 96:128], v1s[t, E], v2s[t, O])
            # bottom rows: -Mi(=mp) | -Mr | Mr | -Mi(=mp)
            nc.vector.tensor_add(wh[b, 0:32], v2s[b, O], v1s[b, E])
            nc.vector.tensor_sub(wh[b, 32:64], v1s[b, O], v2s[b, E])
            nc.gpsimd.tensor_sub(wh[b, 64:96], v2s[b, E], v1s[b, O])
            nc.gpsimd.tensor_add(wh[b, 96:128], v2s[b, O], v1s[b, E])

            # PE warm-keepers bridge the W-build gap and push the ramp
            # counter past 3 us so the T-chain starts at full clock
            with tc.tile_pool(name="wpsum", bufs=1, space="PSUM") as wpsum:
                warm_ps = wpsum.tile([64, 64], fp32, tag="warm")
                for w in range(8):
                    nc.tensor.matmul(warm_ps[:], lhsT=c_sb[:, 0:64],
                                     rhs=acc[:, 0:64], start=True, stop=True)

            # ---- phase 3: e = g .* (W^T g) chunks stream straight to DRAM;
            # the host does the final (tiny) column sums. e chunks are
            # grouped several-per-dma_start (each trigger costs ~0.7 us
            # serially on its issuing sequencer, so 16 separate ones would
            # dominate the tail); the last group is a single chunk so the
            # final DMA fires right after the last multiply. ----
            with tc.tile_pool(name="tpsum", bufs=6, space="PSUM") as tpsum:
                groups = [3, 4, 4, 4, 1]
                ci = 0
                for g, gsz in enumerate(groups):
                    e_big = epool.tile([128, 6 * FCHUNK], fp16, tag="e",
                                       name=f"e{g}")
                    for j in range(gsz):
                        fs = slice(ci * FCHUNK, (ci + 1) * FCHUNK)
                        t_ps = tpsum.tile([128, FCHUNK], fp32, tag="t",
                                          name=f"t{ci}")
                        nc.tensor.matmul(t_ps[:], lhsT=wh[:],
                                         rhs=gth2[0:64, fs],
                                         start=True, stop=True)
                        es = slice(j * FCHUNK, (j + 1) * FCHUNK)
                        if ci not in (2, 5, 8, 11, 13):
                            nc.vector.tensor_mul(e_big[:, es], gth2[:, fs],
                                                 t_ps[:])
                        else:
                            # stage T to SBUF on Act so the Pool engine (no
                            # PSUM access) can handle part of the work
                            t_sb = tsb_pool.tile([128, FCHUNK], fp16,
                                                 tag="tsb", name=f"tsb{ci}")
                            nc.scalar.copy(t_sb[:], t_ps[:])
                            nc.gpsimd.tensor_mul(e_big[:, es], gth2[:, fs],
                                                 t_sb[:])
                        ci += 1
                    fsg = slice((ci - gsz) * FCHUNK, ci * FCHUNK)
                    eng = nc.sync if g % 2 == 0 else nc.scalar
                    eng.dma_start(out_d[:, fsg], e_big[:, 0:gsz * FCHUNK])

    nc.compile()
    return nc


def _prep_g(fbv):
    """gth (64, F) fp16: sqrt(SCALE) * [fbv_re.T; fbv_im.T]."""
    fbv32 = np.ascontiguousarray(fbv).view(np.float32).reshape(F, 2 * R)
    gbt = np.concatenate([fbv32[:, 0::2].T, fbv32[:, 1::2].T], axis=0)
    return (gbt * np.float32(GSCALE)).astype(np.float16)


def _shard_h(arr, core):
    """Core's complex64 shard -> fp16 (4, 128, 2*MACRO_COLS)."""
    sh = arr[core * DIR_PER_CORE:(core + 1) * DIR_PER_CORE]
    f32 = np.ascontiguousarray(sh).view(np.float32)
    return f32.astype(np.float16).reshape(4, 128, 2 * MACRO_COLS)


def _build_in_maps(attenuation_vectors, radiation_vectors,
                   frequency_basis_vectors):
    gth = _prep_g(frequency_basis_vectors)
    consts = _build_consts()
    in_maps = []
    for c in range(N_CORES):
        in_maps.append({
            "rad": _shard_h(radiation_vectors, c),
            "att": _shard_h(attenuation_vectors, c),
            "gth": gth,
            "consts": consts,
        })
    return in_maps


def kernel(attenuation_vectors, radiation_vectors, frequency_basis_vectors):
    from concourse.bass_utils import run_bass_kernel_spmd

    if "nc" not in _NC_CACHE:
        _NC_CACHE["nc"] = build_nc()
    nc = _NC_CACHE["nc"]

    in_maps = _build_in_maps(attenuation_vectors, radiation_vectors,
                             frequency_basis_vectors)
    res = run_bass_kernel_spmd(nc, in_maps, core_ids=list(range(N_CORES)))
    etot = np.zeros((128, F), np.float64)
    for r in res.results:
        etot += r["eout"].astype(np.float64)
    return (etot[0:64].sum(axis=0)
            + 1j * etot[64:128].sum(axis=0)).astype(np.complex64)

